# revision 10
# baseline (speedup 1.0000x reference)
"""Local causal (sliding-window) attention on 8 Trainium2 NeuronCores.

Strategy: sequence-parallel, fully transposed dataflow (features on
partitions, tokens on the free dim). Each core owns 512 consecutive query
tokens of one batch element (cores 0-3 -> batch 0, cores 4-7 -> batch 1)
plus a 128-token halo whose k/v are recomputed locally, so no inter-core
communication is needed.

All matmul inputs are bfloat16 (converted on host): halves HBM traffic vs
f32, runs the 128-wide attention matmuls at full PE rate (f32r drops to 1/4
rate below 256 moving columns), and enables fast weight load. PSUM
accumulation stays f32; softmax reciprocals are computed in f32. The output
is staged to DRAM in bf16 and widened to f32 on the host.

Attention per (supertile st of 256 queries, head h): the 256-query window
spans 3 key blocks of 128 tokens; fully-masked quadrants are never computed
(half-width matmuls). Engine split per head: ScalarE does the exp (PSUM ->
bf16 SBUF) and the denominator-row pack, GpSimd applies the 0/1 window mask
and the final normalizing multiply, DVE copies raw attention out of PSUM
and runs one batched reciprocal per supertile. Denominators ride an extra
ones-column in v through the AV matmul; a one-hot-selector K=16 matmul
broadcasts each head's reciprocal row across partitions.

Scheduling: the two supertiles' 32 heads form one continuous PE pipeline;
v-projection blocks 3,4 are pushed into supertile 0's attention window and
supertile 0's output projection is woven into supertile 1's attention as a
dependency-gated wavefront, so the PE stays dense (HAM stays un-throttled)
from first weight arrival to the tail. Input DMAs split across both HWDGE
rings (sync + scalar) with per-k first blocks for a fast ramp.
"""

import sys

sys.path.insert(0, "/opt/trn_rl_repo")
import numpy as np
import ml_dtypes

BF16 = ml_dtypes.bfloat16

B, S, D = 2, 2048, 1024
H, DH = 16, 64
WINDOW = 128
NCORES = 8
SLOC = 512            # queries per core
HALO = 128
TLOC = SLOC + HALO    # 640 local tokens (halo + queries)
NST = 2               # query supertiles of 256 per core
CPB = NCORES // B     # cores per batch element

_cached = {}


def _build():
    import concourse.bacc as bacc
    import concourse.mybir as mybir
    import concourse.tile as tile

    f32 = mybir.dt.float32
    bf16 = mybir.dt.bfloat16
    AF = mybir.ActivationFunctionType

    nc = bacc.Bacc(None)
    xT_d = nc.declare_dram_parameter("xT", [D, TLOC], bf16, isOutput=False)
    wqkv_d = nc.declare_dram_parameter("w_qkv", [D, 3 * D], bf16, isOutput=False)
    wout_d = nc.declare_dram_parameter("w_out", [D, D], bf16, isOutput=False)
    mask_d = nc.declare_dram_parameter("mask", [NST, 128, 512], bf16, isOutput=False)
    sel_d = nc.declare_dram_parameter("sel", [16, 16 * 128], bf16, isOutput=False)
    out_d = nc.declare_dram_parameter("outT", [D, SLOC], bf16, isOutput=True)

    with tile.TileContext(nc) as tc:
        with (
            tc.tile_pool(name="sb", bufs=1) as sb,
            tc.tile_pool(name="qkps", bufs=1, space="PSUM") as qkps,
            tc.tile_pool(name="scps", bufs=1, space="PSUM") as scps,
            tc.tile_pool(name="aops", bufs=1, space="PSUM") as aops,
            tc.tile_pool(name="pops", bufs=1, space="PSUM") as pops,
        ):
            # ---- resident SBUF tensors; DMA order = consumption order.
            # Ring split: bulk weights + outputs ride the scalar-issued HWDGE
            # ring, x/q-ramp/k/v weights ride the sync ring.
            wqb = [sb.tile([128, 8 * 512], bf16, tag=f"wqb{cb}", name=f"wqb{cb}")
                   for cb in range(6)]
            xt = [sb.tile([128, TLOC], bf16, tag=f"xt{k}", name=f"xt{k}") for k in range(8)]

            def wq_dma(cb, eng):
                eng.dma_start(
                    out=wqb[cb].rearrange("p (k c) -> p k c", k=8),
                    in_=wqkv_d[:, cb * 512:(cb + 1) * 512].rearrange("(k p) c -> p k c", k=8),
                )

            # fast ramp: first q block arrives per-k so the k-loop can start
            # as soon as slice 0 lands
            nc.sync.dma_start(out=xt[0][:], in_=xT_d[0:128, :])
            for k in range(8):
                nc.sync.dma_start(
                    out=wqb[0][:, k * 512:(k + 1) * 512],
                    in_=wqkv_d[k * 128:(k + 1) * 128, 0:512],
                )
                if k < 7:
                    nc.sync.dma_start(out=xt[k + 1][:], in_=xT_d[(k + 1) * 128:(k + 2) * 128, :])
            for cb in range(1, 6):
                wq_dma(cb, nc.sync)
            msk = [sb.tile([128, 512], bf16, tag=f"mk{i}", name=f"mk{i}") for i in range(NST)]
            sel = sb.tile([16, 16 * 128], bf16, tag="sel", name="sel")
            wo = sb.tile([128, 8 * 1024], bf16, tag="wo", name="wo")
            for st in range(NST):
                nc.scalar.dma_start(out=msk[st][:], in_=mask_d[st])
            nc.scalar.dma_start(out=sel[:], in_=sel_d[:])
            nc.scalar.dma_start(
                out=wo.rearrange("p (k c) -> p k c", k=8),
                in_=wout_d.rearrange("(k p) c -> p k c", k=8),
            )

            qT = [sb.tile([128, SLOC], bf16, tag=f"qT{i}", name=f"qT{i}") for i in range(8)]
            kT = [sb.tile([128, TLOC], bf16, tag=f"kT{i}", name=f"kT{i}") for i in range(8)]
            vt = [sb.tile([128, 65 * H], bf16, tag=f"v{t}", name=f"v{t}") for t in range(5)]
            att = [sb.tile([128, SLOC], bf16, tag=f"at{t}", name=f"at{t}") for t in range(8)]
            for t in range(5):
                nc.vector.memset(vt[t].rearrange("p (h c) -> p h c", c=65)[:, :, 64], 1.0)
            scat = [sb.tile([1, H * 256], f32, tag=f"scat{st}", name=f"scat{st}") for st in range(NST)]
            s16 = [sb.tile([16, 256], f32, tag=f"s16_{st}", name=f"s16_{st}") for st in range(NST)]
            r16f = [sb.tile([16, 256], f32, tag=f"r16f_{st}", name=f"r16f_{st}") for st in range(NST)]
            r16b = [sb.tile([16, 256], bf16, tag=f"r16b_{st}", name=f"r16b_{st}") for st in range(NST)]

            # ---- phase 1: qkv projection (v blocks 3,4 deferred into the
            # attention window to keep the PE dense there) ----
            for cb in range(2):            # q columns; queries only
                for m in range(4):
                    ps = qkps.tile([128, 512], f32, tag="qk", bufs=2, name=f"psq{cb}_{m}")
                    for k in range(8):
                        nc.tensor.matmul(
                            ps[:], wqb[cb][:, k * 512 + m * 128:k * 512 + (m + 1) * 128],
                            xt[k][:, HALO:TLOC],
                            start=(k == 0), stop=(k == 7),
                        )
                    nc.scalar.copy(qT[cb * 4 + m][:], ps[:])
            for cb in range(2, 4):         # k columns; all 640 tokens
                for m in range(4):
                    for n in range(2):
                        ps = qkps.tile([128, 320], f32, tag="qk", bufs=2, name=f"psk{cb}_{m}_{n}")
                        for k in range(8):
                            nc.tensor.matmul(
                                ps[:], wqb[cb][:, k * 512 + m * 128:k * 512 + (m + 1) * 128],
                                xt[k][:, n * 320:(n + 1) * 320],
                                start=(k == 0), stop=(k == 7),
                            )
                        nc.scalar.copy(kT[(cb - 2) * 4 + m][:, n * 320:(n + 1) * 320], ps[:])

            def emit_v(t, half):
                # x block stationary so tokens land on partitions
                ps = qkps.tile([128, 512], f32, tag="qk", bufs=2, name=f"psv{t}_{half}")
                for k in range(8):
                    nc.tensor.matmul(
                        ps[:], xt[k][:, t * 128:(t + 1) * 128],
                        wqb[4 + half][:, k * 512:(k + 1) * 512],
                        start=(k == 0), stop=(k == 7),
                    )
                h0 = half * 8
                dst = vt[t].rearrange("p (h c) -> p h c", c=65)[:, h0:h0 + 8, 0:64]
                src = ps[:].rearrange("p (h c) -> p h c", c=64)
                nc.scalar.copy(dst, src)

            for t in range(3):
                for half in range(2):
                    emit_v(t, half)

            # ---- phase 2+3: attention + interleaved output projection ----
            DEPTH = 3
            pend = {}

            def emit_qk(st, h):
                t, poff = h // 2, (h % 2) * 64
                jb, q0 = st * 2, st * 256
                sc = scps.tile([128, 512], f32, tag="sc", bufs=2, name=f"sc_{st}_{h}")
                nc.tensor.matmul(
                    sc[:, 0:128],
                    kT[t][poff:poff + 64, jb * 128:(jb + 1) * 128],
                    qT[t][poff:poff + 64, q0:q0 + 128],
                    start=True, stop=True, skip_group_check=True,
                )
                nc.tensor.matmul(
                    sc[:, 128:256],
                    kT[t][poff:poff + 64, (jb + 2) * 128:(jb + 3) * 128],
                    qT[t][poff:poff + 64, q0 + 128:q0 + 256],
                    start=True, stop=True, skip_group_check=True,
                )
                nc.tensor.matmul(
                    sc[:, 256:512],
                    kT[t][poff:poff + 64, (jb + 1) * 128:(jb + 2) * 128],
                    qT[t][poff:poff + 64, q0:q0 + 256],
                    start=True, stop=True, skip_group_check=True,
                )
                p = sb.tile([128, 512], bf16, tag="pp", bufs=DEPTH + 3, name=f"p_{st}_{h}")
                nc.scalar.activation(p[:], sc[:], AF.Exp, scale=0.125)
                nc.gpsimd.tensor_mul(p[:], p[:], msk[st][:])
                pend[(st, h)] = p

            def emit_av(st, h):
                t, poff = h // 2, (h % 2) * 64
                jb, q0 = st * 2, st * 256
                p = pend.pop((st, h))
                av = aops.tile([65, 256], f32, tag="ao", bufs=2, name=f"av{st}_{h}")
                nc.tensor.matmul(
                    av[:], vt[jb + 1][:, h * 65:h * 65 + 65], p[:, 256:512],
                    start=True, stop=False, skip_group_check=True,
                )
                nc.tensor.matmul(
                    av[:, 0:128], vt[jb][:, h * 65:h * 65 + 65], p[:, 0:128],
                    start=False, stop=False, skip_group_check=True,
                )
                nc.tensor.matmul(
                    av[:, 128:256], vt[jb + 2][:, h * 65:h * 65 + 65], p[:, 128:256],
                    start=False, stop=True, skip_group_check=True,
                )
                nc.scalar.copy(scat[st][0:1, h * 256:(h + 1) * 256], av[64:65, :])
                nc.sync.dma_start(
                    out=s16[st][h:h + 1, :], in_=scat[st][0:1, h * 256:(h + 1) * 256]
                )
                nc.vector.tensor_copy(att[t][poff:poff + 64, q0:q0 + 256], av[0:64, :])

            def emit_recip(st):
                nc.vector.reciprocal(r16f[st][:], s16[st][:])
                nc.scalar.copy(r16b[st][:], r16f[st][:])

            def emit_norm(st, h):
                t, poff = h // 2, (h % 2) * 64
                q0 = st * 256
                rb = qkps.tile([128, 256], f32, tag="qk", bufs=2, name=f"rb{st}_{h}")
                nc.tensor.matmul(
                    rb[:], sel[:, h * 128:(h + 1) * 128], r16b[st][:],
                    start=True, stop=True, skip_group_check=True,
                )
                asl = att[t][poff:poff + 64, q0:q0 + 256]
                nc.vector.tensor_mul(asl, asl, rb[poff:poff + 64, :])

            po_tile = {}

            def emit_po_unit(st, m, k):
                q0 = st * 256
                if k == 0:
                    po_tile[(st, m)] = pops.tile(
                        [128, 256], f32, tag="po", bufs=2, name=f"po{st}_{m}"
                    )
                po = po_tile[(st, m)]
                nc.tensor.matmul(
                    po[:], wo[:, k * 1024 + m * 128:k * 1024 + (m + 1) * 128],
                    att[k][:, q0:q0 + 256],
                    start=(k == 0), stop=(k == 7), skip_group_check=True,
                )
                if k == 7:
                    ot = sb.tile([128, 256], bf16, tag="ot", bufs=4, name=f"ot{st}_{m}")
                    nc.scalar.copy(ot[:], po[:])
                    nc.scalar.dma_start(
                        out=out_d[m * 128:(m + 1) * 128, q0:q0 + 256], in_=ot[:],
                    )

            # unified pipeline: 32 heads; st0 norms + st0 out-proj wavefront
            # and deferred v blocks 3,4 fill supertile boundaries
            po_queue = [(m, k) for m in range(8) for k in range(8)]
            po_ptr = 0
            norm_emitted = -1
            vq = [(3, 0), (3, 1), (4, 0), (4, 1)]
            for step in range(2 * H + DEPTH):
                if step in (1, 3, 5, 7):
                    emit_v(*vq[(step - 1) // 2])
                if step < 2 * H:
                    emit_qk(step // H, step % H)
                s = step - DEPTH
                if s >= 0:
                    emit_av(s // H, s % H)
                    if s == H - 1:
                        emit_recip(0)
                ns = step - (H + DEPTH)
                if 0 <= ns < 8:
                    emit_norm(0, 2 * ns)
                    emit_norm(0, 2 * ns + 1)
                    norm_emitted = 2 * ns + 1
                if step > H + DEPTH:
                    drained = 0
                    while po_ptr < 64 and drained < 6:
                        m, k = po_queue[po_ptr]
                        if 2 * k + 1 <= norm_emitted:
                            emit_po_unit(0, m, k)
                            po_ptr += 1
                            drained += 1
                        else:
                            break
            while po_ptr < 64:
                emit_po_unit(0, *po_queue[po_ptr])
                po_ptr += 1
            emit_recip(1)
            po_ptr = 0
            for j in range(8):
                emit_norm(1, 2 * j)
                emit_norm(1, 2 * j + 1)
                while po_ptr < 64:
                    m, k = po_queue[po_ptr]
                    if k <= j:
                        emit_po_unit(1, m, k)
                        po_ptr += 1
                    else:
                        break
            while po_ptr < 64:
                emit_po_unit(1, *po_queue[po_ptr])
                po_ptr += 1

    nc.finalize()
    return nc


def _get_nc():
    if "nc" not in _cached:
        _cached["nc"] = _build()
    return _cached["nc"]


def _core_inputs(x, w_qkv, w_out):
    wq_b = np.ascontiguousarray(w_qkv.astype(BF16))
    wo_b = np.ascontiguousarray(w_out.astype(BF16))
    sel = np.zeros((16, 16 * 128), dtype=BF16)
    for h in range(H):
        sel[h, h * 128:(h + 1) * 128] = 1
    in_maps = []
    for c in range(NCORES):
        b, qs = c // CPB, (c % CPB) * SLOC
        xs = np.zeros((TLOC, D), dtype=np.float32)
        lo = max(0, qs - HALO)
        xs[HALO - (qs - lo):] = x[b, lo:qs + SLOC]
        # multiplicative 0/1 mask applied to exp(scores) on GpSimd.
        # mask[st][:, 0:128] covers [r0 x queries 0:128], [:, 128:256] covers
        # [r2 x queries 128:256], [:, 256:512] is r1 for all 256 queries.
        i = np.arange(256)[None, None, None, :]
        j = np.arange(128)[None, None, :, None]
        st = np.arange(NST)[:, None, None, None]
        r = np.arange(3)[None, :, None, None]
        qg = qs + st * 256 + i
        kg = qs + st * 256 - HALO + r * 128 + j
        allowed = (kg <= qg) & (kg > qg - WINDOW) & (kg >= 0)
        m3 = allowed.astype(np.float32)
        mask = np.empty((NST, 128, 512), dtype=np.float32)
        mask[:, :, 0:128] = m3[:, 0, :, 0:128]
        mask[:, :, 128:256] = m3[:, 2, :, 128:256]
        mask[:, :, 256:512] = m3[:, 1]
        in_maps.append(
            {
                "xT": np.ascontiguousarray(xs.T.astype(BF16)),
                "w_qkv": wq_b,
                "w_out": wo_b,
                "mask": mask.astype(BF16),
                "sel": sel,
            }
        )
    return in_maps


def kernel(x, w_qkv, w_out, _trace=False, _trace_kwargs=None):
    from concourse.bass_utils import run_bass_kernel_spmd

    x = np.asarray(x, dtype=np.float32)
    w_qkv = np.asarray(w_qkv, dtype=np.float32)
    w_out = np.asarray(w_out, dtype=np.float32)
    nc = _get_nc()
    in_maps = _core_inputs(x, w_qkv, w_out)
    res = run_bass_kernel_spmd(
        nc, in_maps, list(range(NCORES)), trace=_trace, **(_trace_kwargs or {})
    )
    out = np.concatenate(
        [res.results[c]["outT"].astype(np.float32).T for c in range(NCORES)], axis=0
    ).reshape(B, S, D)
    if _trace:
        return out, res
    return out
